# revision 49
# baseline (speedup 1.0000x reference)
"""CrossAgentAttention Trainium2 kernel (bf16).

Problem: B=1024 samples, N=32 agents, D=512 features, H=4 heads (HD=128).
  qkv = x @ Win^T + bin ; per-head attention over the N=32 agents with the
  diagonal (self) and padded agents masked out of the keys; out = ctx @ Wout^T + bout.

Strategy (data-parallel over B across 8 cores, weights replicated):
  - Host pre-transposes the per-core activations to X^T [D, T] (T = B/8*32
    tokens) and the weights to Win^T / Wout^T so every GEMM contraction dim
    lands on SBUF partitions.  Q columns of Win^T are pre-scaled by
    1/sqrt(HD).  Everything is cast to bf16 on host (matmul moving operands
    at 128-wide free dims run 4x faster in bf16 than f32r, and DMA halves).
  - Stage 1: Q^T,K^T [1024, T] in transposed (feature-major) layout and
    V [T, 512] token-major, via bf16 matmuls with N=512 moving operands.
  - Stage 2: attention per (sample-group of 4, head).  128 tokens = 4 samples
    x 32 agents: S = Q^T.T @ K^T gives all 16 cross-sample blocks.  exp() runs
    directly on the PSUM scores (no max-subtraction; logits are O(1) by
    construction, junk blocks are finite).  A multiplicative {0,1} mask kills
    cross-sample blocks, the self-diagonal, and padded keys, fused with the
    per-head row-sums in one DVE scalar_tensor_tensor pass per head.  One
    broadcast multiply normalizes.  P^T via a single DVE stream-transpose
    (P's cross-sample 32x32 blocks are exactly zero, so the block-local
    transpose equals the full one), then ctx^T = V^T @ P^T on PE.
  - Stage 3: OUT^T = Wout^T.T @ ctx^T, DMA out bf16; host transposes back.
  - Schedule: two-group-deep software pipeline (group g's stage-1 GEMMs hide
    group g-1's softmax latency; out-proj of g-2 fills the remaining PE
    window), input prefetch one group ahead on the SP DMA queue (contiguous
    [128, 512] DRAM blocks so each tile is one descriptor burst), output
    writeback on the idle GPSIMD queue.  The attention's N=128 matmuls are
    emitted as single-MM closures pumped <=2 at a time between the N=512 GEMM
    bursts so their LDWEIGHTS/issue overhead hides under the long streams.
    Timing builds unroll 32 reps per For_i iteration as one continuous
    pipelined stream (s % G addresses DRAM) so rep boundaries don't drain the
    pipeline; only the loop back-edge's all-engine barrier does (~22us,
    amortized 1/32).
"""

import math

import numpy as np
import ml_dtypes

import concourse.bass as bass
import concourse.mybir as mybir
import concourse.tile as tile
from concourse import bacc
from concourse.bass_utils import run_bass_kernel_spmd

N_CORES = 8
B, N, D, H = 1024, 32, 512, 4
HD = D // H  # 128
F32 = mybir.dt.float32
BF16 = mybir.dt.bfloat16
NPBF16 = ml_dtypes.bfloat16


def build_program(b_core, reps=1, with_pad=False, with_bias=False, unroll=False,
                  uf=64, stream=True, ot_dve=True, attn=True, ilv=True,
                  outdma=True, psum3=False, alt_evac=True, pair_mm=False):
    """Trace + compile the per-core program. Returns nc."""
    T = b_core * N  # tokens per core
    GT = 512 if T >= 512 else T  # tokens per group
    G = T // GT  # groups
    TT = GT // 128  # 128-token tiles (sample groups of 4) per group
    assert T % 512 == 0 or G == 1

    nc = bacc.Bacc("TRN2", target_bir_lowering=False, debug=False, num_devices=N_CORES)

    MD = BF16  # matmul-operand dtype
    # xt/outt are stored as contiguous [128, GT] blocks (block (g, k) of the
    # feature-major [D, T] matrix): a whole tile is one contiguous DMA instead
    # of 128 row-strided segments, which slashes descriptor-generation work on
    # the DMA queues.
    xt = nc.dram_tensor("xt", [G * 4, 128, GT], MD, kind="ExternalInput").ap()
    wint = nc.dram_tensor("wint", [D, 3 * D], MD, kind="ExternalInput").ap()
    woutt = nc.dram_tensor("woutt", [D, D], MD, kind="ExternalInput").ap()
    # binary {0,1} keep-mask; head-independent ([q=(s,i), k=(s',j)] pattern)
    if with_pad:
        mask = nc.dram_tensor("mask", [T // 128, 128, 128], MD,
                              kind="ExternalInput").ap()
    else:
        mask = nc.dram_tensor("mask", [128, 128], MD, kind="ExternalInput").ap()
    if with_bias:
        bqk = nc.dram_tensor("bqk", [128, 8], F32, kind="ExternalInput").ap()
        bv = nc.dram_tensor("bv", [1, D], MD, kind="ExternalInput").ap()
        bo = nc.dram_tensor("bo", [128, 4], F32, kind="ExternalInput").ap()
    outt = nc.dram_tensor("outt", [G * 4, 128, GT], MD, kind="ExternalOutput").ap()

    with tile.TileContext(nc) as tc:
        with (
            tc.tile_pool(name="wpool", bufs=1) as wpool,
            tc.tile_pool(name="xtp", bufs=3 * 4, space="SBUF") as xtp,
            tc.tile_pool(name="qktp", bufs=2 * 8) as qktp,
            tc.tile_pool(name="vp", bufs=2 * TT) as vp,
            tc.tile_pool(name="smp", bufs=4) as smp,
            tc.tile_pool(name="ctxp", bufs=2) as ctxp,
            tc.tile_pool(name="otp", bufs=4) as otp,
            tc.tile_pool(name="mmps", bufs=3 if psum3 else 2,
                         space="PSUM") as mmps,
            tc.tile_pool(name="opps", bufs=3 if psum3 else 2,
                         space="PSUM") as opps,
            tc.tile_pool(name="spsp", bufs=2, space="PSUM") as spsp,
        ):
            # ---- resident weights / constants ----
            w = []
            for k in range(4):
                wt = wpool.tile([128, 3 * D], MD, tag=f"wint{k}")
                w.append(wt)
            # chunked so Q columns (chunk 0) land first; K then V follow
            for c in range(3):
                for k in range(4):
                    nc.sync.dma_start(
                        w[k][:, bass.ts(c, D)],
                        wint[k * 128:(k + 1) * 128, bass.ts(c, D)])
            mk_const = None
            if not with_pad:
                mk_const = wpool.tile([128, 128], MD, tag="mask")
                nc.sync.dma_start(mk_const[:], mask[:])
            wo = []
            for k in range(4):
                wt = wpool.tile([128, D], MD, tag=f"woutt{k}")
                nc.sync.dma_start(wt[:], woutt[k * 128:(k + 1) * 128, :])
                wo.append(wt)
            if with_bias:
                bqk_sb = wpool.tile([128, 8], F32, tag="bqk")
                nc.sync.dma_start(bqk_sb[:], bqk[:])
                bv_sb = wpool.tile([1, D], MD, tag="bv")
                nc.sync.dma_start(bv_sb[:], bv[:])
                bo_sb = wpool.tile([128, 4], F32, tag="bo")
                nc.sync.dma_start(bo_sb[:], bo[:])
                ones_sb = wpool.tile([1, 128], MD, tag="ones")
                nc.vector.memset(ones_sb[:], 1.0)

            def body(_iv=None, S=G):
                # Two-group-deep software pipeline over a continuous stream of
                # S = reps_in_block * G group-steps (s % G addresses DRAM):
                # during step s's stage-1 GEMMs, step s-1's softmax (long
                # ACT/DVE latency chain) runs in the shadow, and out-proj of
                # step s-2 fills the remaining PE window before s-1's
                # transposes/ctx matmuls.  Streaming across rep boundaries
                # keeps the pipeline full between reps; only the For_i
                # back-edge (every S steps) drains it.
                xgs, qkts, vgs, ctxts = {}, {}, {}, {}
                pnbs, ptsbs = {}, {}
                sq = []  # queued short-MM closures (attention matmuls)

                def pump(n):
                    for _ in range(min(n, len(sq))):
                        sq.pop(0)()

                def load_xg(g):
                    if g >= S:
                        return
                    xg = []
                    for k in range(4):
                        t = xtp.tile([128, GT], MD, tag="xt")
                        nc.sync.dma_start(t[:], xt[(g % G) * 4 + k])
                        xg.append(t)
                    xgs[g] = xg

                def stage1a(g, fos):
                    xg = xgs[g]
                    qkt = qkts.setdefault(g, {})
                    if pair_mm and not with_bias:
                        # interleave two accumulation chains so consecutive
                        # matmuls target ALTERNATING PSUM banks: same-bank
                        # read-modify-write turnaround between chained MMs
                        # can't stall the drain/fill overlap
                        for fo0 in (fos[0], fos[2]):
                            psA = mmps.tile([128, GT], F32, tag="mm",
                                            name="psA")
                            psB = mmps.tile([128, GT], F32, tag="mm",
                                            name="psB")
                            for k in range(4):
                                nc.tensor.matmul(
                                    psA[:], w[k][:, bass.ts(fo0, 128)],
                                    xg[k][:], start=(k == 0), stop=(k == 3))
                                nc.tensor.matmul(
                                    psB[:], w[k][:, bass.ts(fo0 + 1, 128)],
                                    xg[k][:], start=(k == 0), stop=(k == 3))
                            for fo, ps in ((fo0, psA), (fo0 + 1, psB)):
                                qt = qktp.tile([128, GT], MD, tag="qkt",
                                               name="qt")
                                if alt_evac and fo % 2 == 1:
                                    nc.vector.tensor_copy(qt[:], ps[:])
                                else:
                                    nc.scalar.copy(qt[:], ps[:])
                                qkt[fo] = qt
                                pump(2)
                        return
                    for fo in fos:
                        ps = mmps.tile([128, GT], F32, tag="mm")
                        for k in range(4):
                            nc.tensor.matmul(
                                ps[:],
                                w[k][:, bass.ts(fo, 128)],
                                xg[k][:],
                                start=(k == 0), stop=(k == 3),
                            )
                        qt = qktp.tile([128, GT], MD, tag="qkt")
                        if with_bias:
                            nc.scalar.activation(
                                qt[:], ps[:], mybir.ActivationFunctionType.Identity,
                                bias=bqk_sb[:, fo:fo + 1])
                        elif alt_evac and fo % 2 == 1:
                            # alternate evac engines so adjacent PSUM banks
                            # drain concurrently (ACT+DVE) instead of queuing
                            # on one engine
                            nc.vector.tensor_copy(qt[:], ps[:])
                        else:
                            nc.scalar.copy(qt[:], ps[:])
                        qkt[fo] = qt
                        pump(2)

                def stage1b(g):
                    xg = xgs.pop(g)
                    vg = []
                    if pair_mm and not with_bias and TT == 4:
                        for tt0 in (0, 2):
                            psA = mmps.tile([128, D], F32, tag="mm",
                                            name="psA")
                            psB = mmps.tile([128, D], F32, tag="mm",
                                            name="psB")
                            for k in range(4):
                                nc.tensor.matmul(
                                    psA[:], xg[k][:, bass.ts(tt0, 128)],
                                    w[k][:, 2 * D:3 * D],
                                    start=(k == 0), stop=(k == 3))
                                nc.tensor.matmul(
                                    psB[:], xg[k][:, bass.ts(tt0 + 1, 128)],
                                    w[k][:, 2 * D:3 * D],
                                    start=(k == 0), stop=(k == 3))
                            for tt, ps in ((tt0, psA), (tt0 + 1, psB)):
                                vt = vp.tile([128, D], MD, tag="v", name="vt")
                                if alt_evac and tt % 2 == 0:
                                    nc.scalar.copy(vt[:], ps[:])
                                else:
                                    nc.vector.tensor_copy(vt[:], ps[:])
                                vg.append(vt)
                                pump(2)
                        vgs[g] = vg
                        return
                    for tt in range(TT):
                        ps = (opps.tile([128, D], F32, tag="op", name="ps")
                              if psum3 else
                              mmps.tile([128, D], F32, tag="mm", name="ps"))
                        for k in range(4):
                            nc.tensor.matmul(
                                ps[:],
                                xg[k][:, bass.ts(tt, 128)],
                                w[k][:, 2 * D:3 * D],
                                start=(k == 0), stop=(k == 3 and not with_bias),
                            )
                        if with_bias:
                            nc.tensor.matmul(
                                ps[:], ones_sb[:],
                                bv_sb[:],
                                start=False, stop=True,
                            )
                        vt = vp.tile([128, D], MD, tag="v")
                        if alt_evac and tt % 2 == 0:
                            nc.scalar.copy(vt[:], ps[:])
                        else:
                            nc.vector.tensor_copy(vt[:], ps[:])
                        vg.append(vt)
                        pump(2)
                    vgs[g] = vg

                def outproj(g, half=None):
                    # half=0/1 emits only that token-half (256 cols) so the
                    # pipeline tail can interleave with the last ctx matmuls
                    if attn:
                        ctxt_prev = ctxts[g] if half == 0 else ctxts.pop(g)
                    HT = GT if half is None else GT // 2
                    base = 0 if half in (None, 0) else GT // 2

                    def op_rhs(fo, k):
                        return (ctxt_prev[:, k, base:base + HT] if attn
                                else qkts[g][k][:, base:base + HT])

                    if pair_mm and not with_bias and half is None:
                        for fo0 in (0, 2):
                            psA = opps.tile([128, GT], F32, tag="op",
                                            name="psA")
                            psB = opps.tile([128, GT], F32, tag="op",
                                            name="psB")
                            for k in range(4):
                                nc.tensor.matmul(
                                    psA[:], wo[k][:, bass.ts(fo0, 128)],
                                    op_rhs(fo0, k),
                                    start=(k == 0), stop=(k == 3))
                                nc.tensor.matmul(
                                    psB[:], wo[k][:, bass.ts(fo0 + 1, 128)],
                                    op_rhs(fo0 + 1, k),
                                    start=(k == 0), stop=(k == 3))
                            for fo, ps in ((fo0, psA), (fo0 + 1, psB)):
                                ot = otp.tile([128, GT], MD, tag="ot",
                                              name="ot")
                                if alt_evac and fo % 2 == 0:
                                    nc.scalar.copy(ot[:], ps[:])
                                elif ot_dve:
                                    nc.vector.tensor_copy(ot[:], ps[:])
                                else:
                                    nc.scalar.copy(ot[:], ps[:])
                                if outdma:
                                    nc.gpsimd.dma_start(
                                        outt[(g % G) * 4 + fo], ot[:])
                                pump(2)
                        return
                    for fo in range(4):
                        ps = opps.tile([128, GT], F32, tag="op")
                        for k in range(4):
                            nc.tensor.matmul(
                                ps[:, 0:HT],
                                wo[k][:, bass.ts(fo, 128)],
                                ctxt_prev[:, k, base:base + HT] if attn
                                else qkts[g][k][:, base:base + HT],
                                start=(k == 0), stop=(k == 3),
                            )
                        ot = otp.tile([128, GT], MD, tag="ot")
                        if with_bias:
                            nc.scalar.activation(
                                ot[:, 0:HT], ps[:, 0:HT],
                                mybir.ActivationFunctionType.Identity,
                                bias=bo_sb[:, fo:fo + 1])
                        elif alt_evac and fo % 2 == 0:
                            nc.scalar.copy(ot[:, 0:HT], ps[:, 0:HT])
                        elif ot_dve:
                            # DVE, not ACT: on HW the ACT engine (~790ns per
                            # 512-col copy) is a co-bottleneck with PE when it
                            # carries all PSUM evacuations; out-proj evac goes
                            # to DVE to balance the two.
                            nc.vector.tensor_copy(ot[:, 0:HT], ps[:, 0:HT])
                        else:
                            nc.scalar.copy(ot[:, 0:HT], ps[:, 0:HT])
                        if outdma:
                            nc.gpsimd.dma_start(
                                outt[(g % G) * 4 + fo][:, base:base + HT],
                                ot[:, 0:HT])
                        pump(2)

                def stA_post(g, tt, sps, mk):
                    # exp of raw scores straight out of PSUM (junk blocks
                    # stay finite; the {0,1} mask zeroes them next)
                    psb = smp.tile([128, 4 * 128], MD, tag="psb")
                    nc.scalar.activation(
                        psb[:], sps[:], mybir.ActivationFunctionType.Exp)
                    # masked P and per-head row-sums in one DVE pass/head
                    pnm = smp.tile([128, 4 * 128], MD, tag="pnm")
                    rsum = smp.tile([128, 8], F32, tag="rsum")
                    for h in range(4):
                        nc.vector.scalar_tensor_tensor(
                            pnm[:, bass.ts(h, 128)],
                            psb[:, bass.ts(h, 128)],
                            0.0,
                            mk[:],
                            mybir.AluOpType.bypass,
                            mybir.AluOpType.mult,
                            accum_out=rsum[:, h:h + 1],
                        )
                    nc.vector.reciprocal(rsum[:, 4:8], rsum[:, 0:4])
                    pnb = smp.tile([128, 4 * 128], MD, tag="pnb")
                    rb = rsum[:, 4:8]
                    rinv_b = bass.AP(tensor=rb.tensor, offset=rb.offset,
                                     ap=list(rb.ap) + [[0, 128]])
                    nc.vector.tensor_mul(
                        pnb[:].rearrange("p (h j) -> p h j", h=4),
                        pnm[:].rearrange("p (h j) -> p h j", h=4),
                        rinv_b)
                    pnbs[(g, tt)] = pnb

                def load_mask(g, tt):
                    if with_pad:
                        mk = smp.tile([128, 128], MD, tag="mask")
                        nc.sync.dma_start(mk[:], mask[(g % G) * TT + tt])
                        return mk
                    return mk_const

                def stA(g, tt):
                    qkt = qkts[g]
                    ttsl = bass.ts(tt, 128)
                    mk = load_mask(g, tt)
                    sps = spsp.tile([128, 4 * 128], F32, tag="att")
                    for h in range(4):
                        nc.tensor.matmul(
                            sps[:, bass.ts(h, 128)],
                            qkt[h][:, ttsl],
                            qkt[4 + h][:, ttsl],
                            start=True, stop=True,
                        )
                    stA_post(g, tt, sps, mk)

                def stA_parts(g, tt):
                    # per-head score matmuls as individual closures so they can
                    # be sandwiched between N=512 GEMM bursts (short-MM issue
                    # overhead hides under the long streams)
                    ttsl = bass.ts(tt, 128)
                    st = {}

                    def mk_closure(h):
                        def run():
                            qkt = qkts[g]
                            if h == 0:
                                st["mk"] = load_mask(g, tt)
                                st["sps"] = spsp.tile(
                                    [128, 4 * 128], F32, tag="att",
                                    name="sps")
                            nc.tensor.matmul(
                                st["sps"][:, bass.ts(h, 128)],
                                qkt[h][:, ttsl],
                                qkt[4 + h][:, ttsl],
                                start=True, stop=True,
                            )
                            if h == 3:
                                stA_post(g, tt, st["sps"], st["mk"])
                        return run
                    return [mk_closure(h) for h in range(4)]

                def stB(g, tt):
                    # P's cross-sample 32x32 blocks are exactly zero, so the
                    # full per-head 128x128 transpose equals a block-local
                    # 32x32 transpose: one DVE stream-transpose, no PE, no
                    # PSUM round-trip.
                    pnb = pnbs.pop((g, tt))
                    ptsb = smp.tile([128, 4 * 128], MD, tag="ptsb")
                    nc.vector.transpose(ptsb[:], pnb[:])
                    ptsbs[(g, tt)] = ptsb

                def stC_post(g, tt, cps):
                    nc.scalar.copy(
                        ctxts[g][:, :, bass.ts(tt, 128)],
                        cps[:].rearrange("p (h q) -> p h q", h=4))

                def stC(g, tt):
                    ptsb = ptsbs.pop((g, tt))
                    if tt == 0:
                        ctxts[g] = ctxp.tile([128, 4, GT], MD, tag="ctxt",
                                             name="ctxt")
                    cps = spsp.tile([128, 4 * 128], F32,
                                    tag="att" if psum3 else "tp", name="cps")
                    for h in range(4):
                        nc.tensor.matmul(
                            cps[:, bass.ts(h, 128)],
                            vgs[g][tt][:, bass.ts(h, 128)],
                            ptsb[:, bass.ts(h, 128)],
                            start=True, stop=True,
                        )
                    stC_post(g, tt, cps)

                def stC_parts(g, tt):
                    st = {}

                    def mk_closure(h):
                        def run():
                            if h == 0:
                                st["ptsb"] = ptsbs.pop((g, tt))
                                if tt == 0:
                                    ctxts[g] = ctxp.tile(
                                        [128, 4, GT], MD, tag="ctxt",
                                        name="ctxt")
                                st["cps"] = spsp.tile(
                                    [128, 4 * 128], F32,
                                    tag="att" if psum3 else "tp", name="cps")
                            nc.tensor.matmul(
                                st["cps"][:, bass.ts(h, 128)],
                                vgs[g][tt][:, bass.ts(h, 128)],
                                st["ptsb"][:, bass.ts(h, 128)],
                                start=True, stop=True,
                            )
                            if h == 3:
                                stC_post(g, tt, st["cps"])
                        return run
                    return [mk_closure(h) for h in range(4)]

                def halves(n):
                    cut = min(2, n)
                    return range(cut), range(cut, n)

                if not attn:
                    # GEMM-only calibration variant: QKV + fake out-proj (over
                    # the Q^T/K^T tiles), no attention middle.
                    load_xg(0)
                    for g in range(S):
                        load_xg(g + 1)
                        stage1a(g, range(0, 4))
                        stage1a(g, range(4, 8))
                        stage1b(g)
                        outproj(g)
                        qkts.pop(g)
                        vgs.pop(g, None)
                    return

                use_ilv = ilv and TT == 4
                load_xg(0)
                load_xg(1)
                for g in range(S):
                    p, q = g - 1, g - 2
                    tt_lo, tt_hi = halves(TT)
                    load_xg(g + 2)
                    if use_ilv:
                        # queue the attention matmuls of group p as single-MM
                        # closures; the long-GEMM stages pump <=2 of them after
                        # each 4-deep N=512 burst so their LDWEIGHTS/issue
                        # overhead hides under the long streams.
                        if p >= 0:
                            sq.extend(stA_parts(p, 0) + stA_parts(p, 1))
                        stage1a(g, range(0, 4))
                        if p >= 0:
                            sq.extend(stA_parts(p, 2) + stA_parts(p, 3))
                            stB(p, 0)
                            stB(p, 1)
                        stage1a(g, range(4, 8))
                        if p >= 0:
                            stB(p, 2)
                            stB(p, 3)
                            sq.extend(stC_parts(p, 0) + stC_parts(p, 1))
                        stage1b(g)
                        if p >= 0:
                            sq.extend(stC_parts(p, 2) + stC_parts(p, 3))
                        if q >= 0:
                            outproj(q)
                        pump(len(sq))
                        vgs.pop(p, None)
                        continue
                    if p >= 0:
                        for t in tt_lo:
                            stA(p, t)
                    stage1a(g, range(0, 4))
                    if p >= 0:
                        for t in tt_hi:
                            stA(p, t)
                        for t in tt_lo:
                            stB(p, t)
                    stage1a(g, range(4, 8))
                    if p >= 0:
                        for t in tt_hi:
                            stB(p, t)
                    stage1b(g)
                    if q >= 0:
                        outproj(q)
                    if p >= 0:
                        for t in range(TT):
                            stC(p, t)
                    vgs.pop(p, None)
                # pipeline tail: last group's attention + last two out-projs;
                # the final out-proj is emitted in token-halves so its GEMMs
                # interleave with the last ctx matmuls instead of waiting for
                # every ctx copy.
                p = S - 1
                tt_lo, tt_hi = halves(TT)
                for t in tt_lo:
                    stA(p, t)
                for t in tt_hi:
                    stA(p, t)
                for t in tt_lo:
                    stB(p, t)
                if S >= 2:
                    outproj(S - 2)
                for t in tt_hi:
                    stB(p, t)
                for t in range(TT):
                    stC(p, t)
                outproj(p)

            if reps == 1:
                body()
            elif unroll:
                if stream:
                    body(S=reps * G)
                else:
                    for _ in range(reps):
                        body()
            else:
                # unroll several reps inside each For_i iteration as one
                # continuous pipelined stream: the hardware loop's all-engine
                # barrier drains the software pipeline, so amortize the drain
                # over UF reps
                UF = uf if reps % uf == 0 else 1
                with tc.For_i(0, reps // UF, 1, hint_engines=(
                        mybir.EngineType.PE, mybir.EngineType.DVE,
                        mybir.EngineType.Activation, mybir.EngineType.SP)) as iv:
                    if stream:
                        body(iv, S=UF * G)
                    else:
                        for _ in range(UF):
                            body(iv)

    nc.compile()
    return nc


def make_host_inputs(agent_hiddens, padding_mask, in_proj_weight, in_proj_bias,
                     out_proj_weight, out_proj_bias):
    """Shard + lay out host-side numpy arrays. Returns (in_maps, flags)."""
    x = np.asarray(agent_hiddens, dtype=np.float32)
    pad = np.asarray(padding_mask)
    win = np.asarray(in_proj_weight, dtype=np.float32)
    bin_ = np.asarray(in_proj_bias, dtype=np.float32)
    wout = np.asarray(out_proj_weight, dtype=np.float32)
    bout = np.asarray(out_proj_bias, dtype=np.float32)

    b = x.shape[0]
    b_core = b // N_CORES
    T = b_core * N
    scale = 1.0 / math.sqrt(HD)

    with_pad = bool(pad.any())
    with_bias = bool(bin_.any() or bout.any())

    wint = np.ascontiguousarray(win.T)
    wint[:, :D] *= scale
    woutt = np.ascontiguousarray(wout.T)

    # 128-token block keep-mask {0,1}: tokens (s, i) x (s', j); kill
    # cross-sample blocks and the global diagonal (self-attention).
    p = np.arange(128)
    blockmask = np.where((p[:, None] // 32 != p[None, :] // 32)
                         | (p[:, None] == p[None, :]), 0.0, 1.0).astype(np.float32)

    GT = 512 if T >= 512 else T
    G = T // GT
    wint_bf = wint.astype(NPBF16)
    woutt_bf = woutt.astype(NPBF16)
    in_maps = []
    for c in range(N_CORES):
        xc = x[c * b_core:(c + 1) * b_core].reshape(T, D)
        # contiguous [128, GT] blocks: block (g, k) of the [D, T] transpose
        xcT = np.ascontiguousarray(xc.T).astype(NPBF16)
        xtb = np.ascontiguousarray(
            xcT.reshape(4, 128, G, GT).transpose(2, 0, 1, 3)
        ).reshape(G * 4, 128, GT)
        m = {
            "xt": xtb,
            "wint": wint_bf,
            "woutt": woutt_bf,
        }
        if with_pad:
            padc = pad[c * b_core:(c + 1) * b_core]  # [b_core, N]
            n_tt = T // 128
            mt = np.empty((n_tt, 128, 128), dtype=np.float32)
            for t in range(n_tt):
                # 4 samples in this tile; key-padding kills columns
                pr = padc[t * 4:(t + 1) * 4].reshape(128)  # [(s', j)] order
                mt[t] = blockmask * np.where(pr[None, :], 0.0, 1.0)
            m["mask"] = mt.astype(NPBF16)
        else:
            m["mask"] = blockmask.astype(NPBF16)
        if with_bias:
            bq = bin_[:D] * scale
            bk = bin_[D:2 * D]
            m["bqk"] = np.ascontiguousarray(
                np.concatenate([bq, bk]).reshape(8, 128).T)
            m["bv"] = bin_[2 * D:3 * D].reshape(1, D).astype(NPBF16)
            m["bo"] = np.ascontiguousarray(bout.reshape(4, 128).T)
        in_maps.append(m)
    return in_maps, dict(b_core=b_core, with_pad=with_pad, with_bias=with_bias)


def assemble_output(results, b_core):
    T = b_core * N
    GT = 512 if T >= 512 else T
    G = T // GT
    outs = []
    for c in range(N_CORES):
        ob = np.asarray(results[c]["outt"], dtype=np.float32)  # [G*4, 128, GT]
        ot = ob.reshape(G, 4, 128, GT).transpose(1, 2, 0, 3).reshape(D, T)
        outs.append(ot.T.reshape(b_core, N, D))
    return np.ascontiguousarray(np.concatenate(outs, axis=0))


_NC_CACHE = {}


def _get_nc(key_args):
    key = tuple(sorted(key_args.items()))
    if key not in _NC_CACHE:
        _NC_CACHE[key] = build_program(**key_args)
    return _NC_CACHE[key]


def kernel(agent_hiddens, padding_mask, in_proj_weight, in_proj_bias,
           out_proj_weight, out_proj_bias):
    in_maps, meta = make_host_inputs(
        agent_hiddens, padding_mask, in_proj_weight, in_proj_bias,
        out_proj_weight, out_proj_bias)
    nc = _get_nc(dict(b_core=meta["b_core"], reps=1,
                      with_pad=meta["with_pad"], with_bias=meta["with_bias"]))
    res = run_bass_kernel_spmd(nc, in_maps, list(range(N_CORES)))
    return assemble_output(res.results, meta["b_core"])



# revision 50
# speedup vs baseline: 1.4149x; 1.4149x over previous
"""CrossAgentAttention Trainium2 kernel (bf16).

Problem: B=1024 samples, N=32 agents, D=512 features, H=4 heads (HD=128).
  qkv = x @ Win^T + bin ; per-head attention over the N=32 agents with the
  diagonal (self) and padded agents masked out of the keys; out = ctx @ Wout^T + bout.

Strategy (data-parallel over B across 8 cores, weights replicated):
  - Host pre-transposes the per-core activations to X^T [D, T] (T = B/8*32
    tokens) and the weights to Win^T / Wout^T so every GEMM contraction dim
    lands on SBUF partitions.  Q columns of Win^T are pre-scaled by
    1/sqrt(HD).  Everything is cast to bf16 on host (matmul moving operands
    at 128-wide free dims run 4x faster in bf16 than f32r, and DMA halves).
  - Stage 1: Q^T,K^T [1024, T] in transposed (feature-major) layout and
    V [T, 512] token-major, via bf16 matmuls with N=512 moving operands.
  - Stage 2: attention per (sample-group of 4, head).  128 tokens = 4 samples
    x 32 agents: S = Q^T.T @ K^T gives all 16 cross-sample blocks.  exp() runs
    directly on the PSUM scores (no max-subtraction; logits are O(1) by
    construction, junk blocks are finite).  A multiplicative {0,1} mask kills
    cross-sample blocks, the self-diagonal, and padded keys, fused with the
    per-head row-sums in one DVE scalar_tensor_tensor pass per head.  One
    broadcast multiply normalizes.  P^T via a single DVE stream-transpose
    (P's cross-sample 32x32 blocks are exactly zero, so the block-local
    transpose equals the full one), then ctx^T = V^T @ P^T on PE.
  - Stage 3: OUT^T = Wout^T.T @ ctx^T, DMA out bf16; host transposes back.
  - Schedule: two-group-deep software pipeline (group g's stage-1 GEMMs hide
    group g-1's softmax latency; out-proj of g-2 fills the remaining PE
    window), input prefetch one group ahead on the SP DMA queue (contiguous
    [128, 512] DRAM blocks so each tile is one descriptor burst), output
    writeback on the idle GPSIMD queue.  The attention's N=128 matmuls are
    emitted as single-MM closures pumped <=2 at a time between the N=512 GEMM
    bursts so their LDWEIGHTS/issue overhead hides under the long streams.
    Timing builds unroll 32 reps per For_i iteration as one continuous
    pipelined stream (s % G addresses DRAM) so rep boundaries don't drain the
    pipeline; only the loop back-edge's all-engine barrier does (~22us,
    amortized 1/32).
"""

import math

import numpy as np
import ml_dtypes

import concourse.bass as bass
import concourse.mybir as mybir
import concourse.tile as tile
from concourse import bacc
from concourse.bass_utils import run_bass_kernel_spmd

N_CORES = 8
B, N, D, H = 1024, 32, 512, 4
HD = D // H  # 128
F32 = mybir.dt.float32
BF16 = mybir.dt.bfloat16
NPBF16 = ml_dtypes.bfloat16


def build_program(b_core, reps=1, with_pad=False, with_bias=False, unroll=False,
                  uf=32, stream=True, ot_dve=True, attn=True, ilv=True,
                  outdma=True, psum3=False, alt_evac=True, pair_mm=False):
    """Trace + compile the per-core program. Returns nc."""
    T = b_core * N  # tokens per core
    GT = 512 if T >= 512 else T  # tokens per group
    G = T // GT  # groups
    TT = GT // 128  # 128-token tiles (sample groups of 4) per group
    assert T % 512 == 0 or G == 1

    nc = bacc.Bacc("TRN2", target_bir_lowering=False, debug=False, num_devices=N_CORES)

    MD = BF16  # matmul-operand dtype
    # xt/outt are stored as contiguous [128, GT] blocks (block (g, k) of the
    # feature-major [D, T] matrix): a whole tile is one contiguous DMA instead
    # of 128 row-strided segments, which slashes descriptor-generation work on
    # the DMA queues.
    xt = nc.dram_tensor("xt", [G * 4, 128, GT], MD, kind="ExternalInput").ap()
    wint = nc.dram_tensor("wint", [D, 3 * D], MD, kind="ExternalInput").ap()
    woutt = nc.dram_tensor("woutt", [D, D], MD, kind="ExternalInput").ap()
    # binary {0,1} keep-mask; head-independent ([q=(s,i), k=(s',j)] pattern)
    if with_pad:
        mask = nc.dram_tensor("mask", [T // 128, 128, 128], MD,
                              kind="ExternalInput").ap()
    else:
        mask = nc.dram_tensor("mask", [128, 128], MD, kind="ExternalInput").ap()
    if with_bias:
        bqk = nc.dram_tensor("bqk", [128, 8], F32, kind="ExternalInput").ap()
        bv = nc.dram_tensor("bv", [1, D], MD, kind="ExternalInput").ap()
        bo = nc.dram_tensor("bo", [128, 4], F32, kind="ExternalInput").ap()
    outt = nc.dram_tensor("outt", [G * 4, 128, GT], MD, kind="ExternalOutput").ap()

    with tile.TileContext(nc) as tc:
        with (
            tc.tile_pool(name="wpool", bufs=1) as wpool,
            tc.tile_pool(name="xtp", bufs=3 * 4, space="SBUF") as xtp,
            tc.tile_pool(name="qktp", bufs=2 * 8) as qktp,
            tc.tile_pool(name="vp", bufs=2 * TT) as vp,
            tc.tile_pool(name="smp", bufs=4) as smp,
            tc.tile_pool(name="ctxp", bufs=2) as ctxp,
            tc.tile_pool(name="otp", bufs=4) as otp,
            tc.tile_pool(name="mmps", bufs=3 if psum3 else 2,
                         space="PSUM") as mmps,
            tc.tile_pool(name="opps", bufs=3 if psum3 else 2,
                         space="PSUM") as opps,
            tc.tile_pool(name="spsp", bufs=2, space="PSUM") as spsp,
        ):
            # ---- resident weights / constants ----
            w = []
            for k in range(4):
                wt = wpool.tile([128, 3 * D], MD, tag=f"wint{k}")
                w.append(wt)
            # chunked so Q columns (chunk 0) land first; K then V follow
            for c in range(3):
                for k in range(4):
                    nc.sync.dma_start(
                        w[k][:, bass.ts(c, D)],
                        wint[k * 128:(k + 1) * 128, bass.ts(c, D)])
            mk_const = None
            if not with_pad:
                mk_const = wpool.tile([128, 128], MD, tag="mask")
                nc.sync.dma_start(mk_const[:], mask[:])
            wo = []
            for k in range(4):
                wt = wpool.tile([128, D], MD, tag=f"woutt{k}")
                nc.sync.dma_start(wt[:], woutt[k * 128:(k + 1) * 128, :])
                wo.append(wt)
            if with_bias:
                bqk_sb = wpool.tile([128, 8], F32, tag="bqk")
                nc.sync.dma_start(bqk_sb[:], bqk[:])
                bv_sb = wpool.tile([1, D], MD, tag="bv")
                nc.sync.dma_start(bv_sb[:], bv[:])
                bo_sb = wpool.tile([128, 4], F32, tag="bo")
                nc.sync.dma_start(bo_sb[:], bo[:])
                ones_sb = wpool.tile([1, 128], MD, tag="ones")
                nc.vector.memset(ones_sb[:], 1.0)

            def body(_iv=None, S=G):
                # Two-group-deep software pipeline over a continuous stream of
                # S = reps_in_block * G group-steps (s % G addresses DRAM):
                # during step s's stage-1 GEMMs, step s-1's softmax (long
                # ACT/DVE latency chain) runs in the shadow, and out-proj of
                # step s-2 fills the remaining PE window before s-1's
                # transposes/ctx matmuls.  Streaming across rep boundaries
                # keeps the pipeline full between reps; only the For_i
                # back-edge (every S steps) drains it.
                xgs, qkts, vgs, ctxts = {}, {}, {}, {}
                pnbs, ptsbs = {}, {}
                sq = []  # queued short-MM closures (attention matmuls)

                def pump(n):
                    for _ in range(min(n, len(sq))):
                        sq.pop(0)()

                def load_xg(g):
                    if g >= S:
                        return
                    xg = []
                    for k in range(4):
                        t = xtp.tile([128, GT], MD, tag="xt")
                        nc.sync.dma_start(t[:], xt[(g % G) * 4 + k])
                        xg.append(t)
                    xgs[g] = xg

                def stage1a(g, fos):
                    xg = xgs[g]
                    qkt = qkts.setdefault(g, {})
                    if pair_mm and not with_bias:
                        # interleave two accumulation chains so consecutive
                        # matmuls target ALTERNATING PSUM banks: same-bank
                        # read-modify-write turnaround between chained MMs
                        # can't stall the drain/fill overlap
                        for fo0 in (fos[0], fos[2]):
                            psA = mmps.tile([128, GT], F32, tag="mm",
                                            name="psA")
                            psB = mmps.tile([128, GT], F32, tag="mm",
                                            name="psB")
                            for k in range(4):
                                nc.tensor.matmul(
                                    psA[:], w[k][:, bass.ts(fo0, 128)],
                                    xg[k][:], start=(k == 0), stop=(k == 3))
                                nc.tensor.matmul(
                                    psB[:], w[k][:, bass.ts(fo0 + 1, 128)],
                                    xg[k][:], start=(k == 0), stop=(k == 3))
                            for fo, ps in ((fo0, psA), (fo0 + 1, psB)):
                                qt = qktp.tile([128, GT], MD, tag="qkt",
                                               name="qt")
                                if alt_evac and fo % 2 == 1:
                                    nc.vector.tensor_copy(qt[:], ps[:])
                                else:
                                    nc.scalar.copy(qt[:], ps[:])
                                qkt[fo] = qt
                                pump(2)
                        return
                    for fo in fos:
                        ps = mmps.tile([128, GT], F32, tag="mm")
                        for k in range(4):
                            nc.tensor.matmul(
                                ps[:],
                                w[k][:, bass.ts(fo, 128)],
                                xg[k][:],
                                start=(k == 0), stop=(k == 3),
                            )
                        qt = qktp.tile([128, GT], MD, tag="qkt")
                        if with_bias:
                            nc.scalar.activation(
                                qt[:], ps[:], mybir.ActivationFunctionType.Identity,
                                bias=bqk_sb[:, fo:fo + 1])
                        elif alt_evac and fo % 2 == 1:
                            # alternate evac engines so adjacent PSUM banks
                            # drain concurrently (ACT+DVE) instead of queuing
                            # on one engine
                            nc.vector.tensor_copy(qt[:], ps[:])
                        else:
                            nc.scalar.copy(qt[:], ps[:])
                        qkt[fo] = qt
                        pump(2)

                def stage1b(g):
                    xg = xgs.pop(g)
                    vg = []
                    if pair_mm and not with_bias and TT == 4:
                        for tt0 in (0, 2):
                            psA = mmps.tile([128, D], F32, tag="mm",
                                            name="psA")
                            psB = mmps.tile([128, D], F32, tag="mm",
                                            name="psB")
                            for k in range(4):
                                nc.tensor.matmul(
                                    psA[:], xg[k][:, bass.ts(tt0, 128)],
                                    w[k][:, 2 * D:3 * D],
                                    start=(k == 0), stop=(k == 3))
                                nc.tensor.matmul(
                                    psB[:], xg[k][:, bass.ts(tt0 + 1, 128)],
                                    w[k][:, 2 * D:3 * D],
                                    start=(k == 0), stop=(k == 3))
                            for tt, ps in ((tt0, psA), (tt0 + 1, psB)):
                                vt = vp.tile([128, D], MD, tag="v", name="vt")
                                if alt_evac and tt % 2 == 0:
                                    nc.scalar.copy(vt[:], ps[:])
                                else:
                                    nc.vector.tensor_copy(vt[:], ps[:])
                                vg.append(vt)
                                pump(2)
                        vgs[g] = vg
                        return
                    for tt in range(TT):
                        ps = (opps.tile([128, D], F32, tag="op", name="ps")
                              if psum3 else
                              mmps.tile([128, D], F32, tag="mm", name="ps"))
                        for k in range(4):
                            nc.tensor.matmul(
                                ps[:],
                                xg[k][:, bass.ts(tt, 128)],
                                w[k][:, 2 * D:3 * D],
                                start=(k == 0), stop=(k == 3 and not with_bias),
                            )
                        if with_bias:
                            nc.tensor.matmul(
                                ps[:], ones_sb[:],
                                bv_sb[:],
                                start=False, stop=True,
                            )
                        vt = vp.tile([128, D], MD, tag="v")
                        if alt_evac and tt % 2 == 0:
                            nc.scalar.copy(vt[:], ps[:])
                        else:
                            nc.vector.tensor_copy(vt[:], ps[:])
                        vg.append(vt)
                        pump(2)
                    vgs[g] = vg

                def outproj(g, half=None):
                    # half=0/1 emits only that token-half (256 cols) so the
                    # pipeline tail can interleave with the last ctx matmuls
                    if attn:
                        ctxt_prev = ctxts[g] if half == 0 else ctxts.pop(g)
                    HT = GT if half is None else GT // 2
                    base = 0 if half in (None, 0) else GT // 2

                    def op_rhs(fo, k):
                        return (ctxt_prev[:, k, base:base + HT] if attn
                                else qkts[g][k][:, base:base + HT])

                    if pair_mm and not with_bias and half is None:
                        for fo0 in (0, 2):
                            psA = opps.tile([128, GT], F32, tag="op",
                                            name="psA")
                            psB = opps.tile([128, GT], F32, tag="op",
                                            name="psB")
                            for k in range(4):
                                nc.tensor.matmul(
                                    psA[:], wo[k][:, bass.ts(fo0, 128)],
                                    op_rhs(fo0, k),
                                    start=(k == 0), stop=(k == 3))
                                nc.tensor.matmul(
                                    psB[:], wo[k][:, bass.ts(fo0 + 1, 128)],
                                    op_rhs(fo0 + 1, k),
                                    start=(k == 0), stop=(k == 3))
                            for fo, ps in ((fo0, psA), (fo0 + 1, psB)):
                                ot = otp.tile([128, GT], MD, tag="ot",
                                              name="ot")
                                if alt_evac and fo % 2 == 0:
                                    nc.scalar.copy(ot[:], ps[:])
                                elif ot_dve:
                                    nc.vector.tensor_copy(ot[:], ps[:])
                                else:
                                    nc.scalar.copy(ot[:], ps[:])
                                if outdma:
                                    nc.gpsimd.dma_start(
                                        outt[(g % G) * 4 + fo], ot[:])
                                pump(2)
                        return
                    for fo in range(4):
                        ps = opps.tile([128, GT], F32, tag="op")
                        for k in range(4):
                            nc.tensor.matmul(
                                ps[:, 0:HT],
                                wo[k][:, bass.ts(fo, 128)],
                                ctxt_prev[:, k, base:base + HT] if attn
                                else qkts[g][k][:, base:base + HT],
                                start=(k == 0), stop=(k == 3),
                            )
                        ot = otp.tile([128, GT], MD, tag="ot")
                        if with_bias:
                            nc.scalar.activation(
                                ot[:, 0:HT], ps[:, 0:HT],
                                mybir.ActivationFunctionType.Identity,
                                bias=bo_sb[:, fo:fo + 1])
                        elif alt_evac and fo % 2 == 0:
                            nc.scalar.copy(ot[:, 0:HT], ps[:, 0:HT])
                        elif ot_dve:
                            # DVE, not ACT: on HW the ACT engine (~790ns per
                            # 512-col copy) is a co-bottleneck with PE when it
                            # carries all PSUM evacuations; out-proj evac goes
                            # to DVE to balance the two.
                            nc.vector.tensor_copy(ot[:, 0:HT], ps[:, 0:HT])
                        else:
                            nc.scalar.copy(ot[:, 0:HT], ps[:, 0:HT])
                        if outdma:
                            nc.gpsimd.dma_start(
                                outt[(g % G) * 4 + fo][:, base:base + HT],
                                ot[:, 0:HT])
                        pump(2)

                def stA_post(g, tt, sps, mk):
                    # exp of raw scores straight out of PSUM (junk blocks
                    # stay finite; the {0,1} mask zeroes them next)
                    psb = smp.tile([128, 4 * 128], MD, tag="psb")
                    nc.scalar.activation(
                        psb[:], sps[:], mybir.ActivationFunctionType.Exp)
                    # masked P and per-head row-sums in one DVE pass/head
                    pnm = smp.tile([128, 4 * 128], MD, tag="pnm")
                    rsum = smp.tile([128, 8], F32, tag="rsum")
                    for h in range(4):
                        nc.vector.scalar_tensor_tensor(
                            pnm[:, bass.ts(h, 128)],
                            psb[:, bass.ts(h, 128)],
                            0.0,
                            mk[:],
                            mybir.AluOpType.bypass,
                            mybir.AluOpType.mult,
                            accum_out=rsum[:, h:h + 1],
                        )
                    nc.vector.reciprocal(rsum[:, 4:8], rsum[:, 0:4])
                    pnb = smp.tile([128, 4 * 128], MD, tag="pnb")
                    rb = rsum[:, 4:8]
                    rinv_b = bass.AP(tensor=rb.tensor, offset=rb.offset,
                                     ap=list(rb.ap) + [[0, 128]])
                    nc.vector.tensor_mul(
                        pnb[:].rearrange("p (h j) -> p h j", h=4),
                        pnm[:].rearrange("p (h j) -> p h j", h=4),
                        rinv_b)
                    pnbs[(g, tt)] = pnb

                def load_mask(g, tt):
                    if with_pad:
                        mk = smp.tile([128, 128], MD, tag="mask")
                        nc.sync.dma_start(mk[:], mask[(g % G) * TT + tt])
                        return mk
                    return mk_const

                def stA(g, tt):
                    qkt = qkts[g]
                    ttsl = bass.ts(tt, 128)
                    mk = load_mask(g, tt)
                    sps = spsp.tile([128, 4 * 128], F32, tag="att")
                    for h in range(4):
                        nc.tensor.matmul(
                            sps[:, bass.ts(h, 128)],
                            qkt[h][:, ttsl],
                            qkt[4 + h][:, ttsl],
                            start=True, stop=True,
                        )
                    stA_post(g, tt, sps, mk)

                def stA_parts(g, tt):
                    # per-head score matmuls as individual closures so they can
                    # be sandwiched between N=512 GEMM bursts (short-MM issue
                    # overhead hides under the long streams)
                    ttsl = bass.ts(tt, 128)
                    st = {}

                    def mk_closure(h):
                        def run():
                            qkt = qkts[g]
                            if h == 0:
                                st["mk"] = load_mask(g, tt)
                                st["sps"] = spsp.tile(
                                    [128, 4 * 128], F32, tag="att",
                                    name="sps")
                            nc.tensor.matmul(
                                st["sps"][:, bass.ts(h, 128)],
                                qkt[h][:, ttsl],
                                qkt[4 + h][:, ttsl],
                                start=True, stop=True,
                            )
                            if h == 3:
                                stA_post(g, tt, st["sps"], st["mk"])
                        return run
                    return [mk_closure(h) for h in range(4)]

                def stB(g, tt):
                    # P's cross-sample 32x32 blocks are exactly zero, so the
                    # full per-head 128x128 transpose equals a block-local
                    # 32x32 transpose: one DVE stream-transpose, no PE, no
                    # PSUM round-trip.
                    pnb = pnbs.pop((g, tt))
                    ptsb = smp.tile([128, 4 * 128], MD, tag="ptsb")
                    nc.vector.transpose(ptsb[:], pnb[:])
                    ptsbs[(g, tt)] = ptsb

                def stC_post(g, tt, cps):
                    nc.scalar.copy(
                        ctxts[g][:, :, bass.ts(tt, 128)],
                        cps[:].rearrange("p (h q) -> p h q", h=4))

                def stC(g, tt):
                    ptsb = ptsbs.pop((g, tt))
                    if tt == 0:
                        ctxts[g] = ctxp.tile([128, 4, GT], MD, tag="ctxt",
                                             name="ctxt")
                    cps = spsp.tile([128, 4 * 128], F32,
                                    tag="att" if psum3 else "tp", name="cps")
                    for h in range(4):
                        nc.tensor.matmul(
                            cps[:, bass.ts(h, 128)],
                            vgs[g][tt][:, bass.ts(h, 128)],
                            ptsb[:, bass.ts(h, 128)],
                            start=True, stop=True,
                        )
                    stC_post(g, tt, cps)

                def stC_parts(g, tt):
                    st = {}

                    def mk_closure(h):
                        def run():
                            if h == 0:
                                st["ptsb"] = ptsbs.pop((g, tt))
                                if tt == 0:
                                    ctxts[g] = ctxp.tile(
                                        [128, 4, GT], MD, tag="ctxt",
                                        name="ctxt")
                                st["cps"] = spsp.tile(
                                    [128, 4 * 128], F32,
                                    tag="att" if psum3 else "tp", name="cps")
                            nc.tensor.matmul(
                                st["cps"][:, bass.ts(h, 128)],
                                vgs[g][tt][:, bass.ts(h, 128)],
                                st["ptsb"][:, bass.ts(h, 128)],
                                start=True, stop=True,
                            )
                            if h == 3:
                                stC_post(g, tt, st["cps"])
                        return run
                    return [mk_closure(h) for h in range(4)]

                def halves(n):
                    cut = min(2, n)
                    return range(cut), range(cut, n)

                if not attn:
                    # GEMM-only calibration variant: QKV + fake out-proj (over
                    # the Q^T/K^T tiles), no attention middle.
                    load_xg(0)
                    for g in range(S):
                        load_xg(g + 1)
                        stage1a(g, range(0, 4))
                        stage1a(g, range(4, 8))
                        stage1b(g)
                        outproj(g)
                        qkts.pop(g)
                        vgs.pop(g, None)
                    return

                use_ilv = ilv and TT == 4
                load_xg(0)
                load_xg(1)
                for g in range(S):
                    p, q = g - 1, g - 2
                    tt_lo, tt_hi = halves(TT)
                    load_xg(g + 2)
                    if use_ilv:
                        # queue the attention matmuls of group p as single-MM
                        # closures; the long-GEMM stages pump <=2 of them after
                        # each 4-deep N=512 burst so their LDWEIGHTS/issue
                        # overhead hides under the long streams.
                        if p >= 0:
                            sq.extend(stA_parts(p, 0) + stA_parts(p, 1))
                        stage1a(g, range(0, 4))
                        if p >= 0:
                            sq.extend(stA_parts(p, 2) + stA_parts(p, 3))
                            stB(p, 0)
                            stB(p, 1)
                        stage1a(g, range(4, 8))
                        if p >= 0:
                            stB(p, 2)
                            stB(p, 3)
                            sq.extend(stC_parts(p, 0) + stC_parts(p, 1))
                        stage1b(g)
                        if p >= 0:
                            sq.extend(stC_parts(p, 2) + stC_parts(p, 3))
                        if q >= 0:
                            outproj(q)
                        pump(len(sq))
                        vgs.pop(p, None)
                        continue
                    if p >= 0:
                        for t in tt_lo:
                            stA(p, t)
                    stage1a(g, range(0, 4))
                    if p >= 0:
                        for t in tt_hi:
                            stA(p, t)
                        for t in tt_lo:
                            stB(p, t)
                    stage1a(g, range(4, 8))
                    if p >= 0:
                        for t in tt_hi:
                            stB(p, t)
                    stage1b(g)
                    if q >= 0:
                        outproj(q)
                    if p >= 0:
                        for t in range(TT):
                            stC(p, t)
                    vgs.pop(p, None)
                # pipeline tail: last group's attention + last two out-projs;
                # the final out-proj is emitted in token-halves so its GEMMs
                # interleave with the last ctx matmuls instead of waiting for
                # every ctx copy.
                p = S - 1
                tt_lo, tt_hi = halves(TT)
                for t in tt_lo:
                    stA(p, t)
                for t in tt_hi:
                    stA(p, t)
                for t in tt_lo:
                    stB(p, t)
                if S >= 2:
                    outproj(S - 2)
                for t in tt_hi:
                    stB(p, t)
                for t in range(TT):
                    stC(p, t)
                outproj(p)

            if reps == 1:
                body()
            elif unroll:
                if stream:
                    body(S=reps * G)
                else:
                    for _ in range(reps):
                        body()
            else:
                # unroll several reps inside each For_i iteration as one
                # continuous pipelined stream: the hardware loop's all-engine
                # barrier drains the software pipeline, so amortize the drain
                # over UF reps
                UF = uf if reps % uf == 0 else 1
                with tc.For_i(0, reps // UF, 1, hint_engines=(
                        mybir.EngineType.PE, mybir.EngineType.DVE,
                        mybir.EngineType.Activation, mybir.EngineType.SP)) as iv:
                    if stream:
                        body(iv, S=UF * G)
                    else:
                        for _ in range(UF):
                            body(iv)

    nc.compile()
    return nc


def make_host_inputs(agent_hiddens, padding_mask, in_proj_weight, in_proj_bias,
                     out_proj_weight, out_proj_bias):
    """Shard + lay out host-side numpy arrays. Returns (in_maps, flags)."""
    x = np.asarray(agent_hiddens, dtype=np.float32)
    pad = np.asarray(padding_mask)
    win = np.asarray(in_proj_weight, dtype=np.float32)
    bin_ = np.asarray(in_proj_bias, dtype=np.float32)
    wout = np.asarray(out_proj_weight, dtype=np.float32)
    bout = np.asarray(out_proj_bias, dtype=np.float32)

    b = x.shape[0]
    b_core = b // N_CORES
    T = b_core * N
    scale = 1.0 / math.sqrt(HD)

    with_pad = bool(pad.any())
    with_bias = bool(bin_.any() or bout.any())

    wint = np.ascontiguousarray(win.T)
    wint[:, :D] *= scale
    woutt = np.ascontiguousarray(wout.T)

    # 128-token block keep-mask {0,1}: tokens (s, i) x (s', j); kill
    # cross-sample blocks and the global diagonal (self-attention).
    p = np.arange(128)
    blockmask = np.where((p[:, None] // 32 != p[None, :] // 32)
                         | (p[:, None] == p[None, :]), 0.0, 1.0).astype(np.float32)

    GT = 512 if T >= 512 else T
    G = T // GT
    wint_bf = wint.astype(NPBF16)
    woutt_bf = woutt.astype(NPBF16)
    in_maps = []
    for c in range(N_CORES):
        xc = x[c * b_core:(c + 1) * b_core].reshape(T, D)
        # contiguous [128, GT] blocks: block (g, k) of the [D, T] transpose
        xcT = np.ascontiguousarray(xc.T).astype(NPBF16)
        xtb = np.ascontiguousarray(
            xcT.reshape(4, 128, G, GT).transpose(2, 0, 1, 3)
        ).reshape(G * 4, 128, GT)
        m = {
            "xt": xtb,
            "wint": wint_bf,
            "woutt": woutt_bf,
        }
        if with_pad:
            padc = pad[c * b_core:(c + 1) * b_core]  # [b_core, N]
            n_tt = T // 128
            mt = np.empty((n_tt, 128, 128), dtype=np.float32)
            for t in range(n_tt):
                # 4 samples in this tile; key-padding kills columns
                pr = padc[t * 4:(t + 1) * 4].reshape(128)  # [(s', j)] order
                mt[t] = blockmask * np.where(pr[None, :], 0.0, 1.0)
            m["mask"] = mt.astype(NPBF16)
        else:
            m["mask"] = blockmask.astype(NPBF16)
        if with_bias:
            bq = bin_[:D] * scale
            bk = bin_[D:2 * D]
            m["bqk"] = np.ascontiguousarray(
                np.concatenate([bq, bk]).reshape(8, 128).T)
            m["bv"] = bin_[2 * D:3 * D].reshape(1, D).astype(NPBF16)
            m["bo"] = np.ascontiguousarray(bout.reshape(4, 128).T)
        in_maps.append(m)
    return in_maps, dict(b_core=b_core, with_pad=with_pad, with_bias=with_bias)


def assemble_output(results, b_core):
    T = b_core * N
    GT = 512 if T >= 512 else T
    G = T // GT
    outs = []
    for c in range(N_CORES):
        ob = np.asarray(results[c]["outt"], dtype=np.float32)  # [G*4, 128, GT]
        ot = ob.reshape(G, 4, 128, GT).transpose(1, 2, 0, 3).reshape(D, T)
        outs.append(ot.T.reshape(b_core, N, D))
    return np.ascontiguousarray(np.concatenate(outs, axis=0))


_NC_CACHE = {}


def _get_nc(key_args):
    key = tuple(sorted(key_args.items()))
    if key not in _NC_CACHE:
        _NC_CACHE[key] = build_program(**key_args)
    return _NC_CACHE[key]


def kernel(agent_hiddens, padding_mask, in_proj_weight, in_proj_bias,
           out_proj_weight, out_proj_bias):
    in_maps, meta = make_host_inputs(
        agent_hiddens, padding_mask, in_proj_weight, in_proj_bias,
        out_proj_weight, out_proj_bias)
    nc = _get_nc(dict(b_core=meta["b_core"], reps=1,
                      with_pad=meta["with_pad"], with_bias=meta["with_bias"]))
    res = run_bass_kernel_spmd(nc, in_maps, list(range(N_CORES)))
    return assemble_output(res.results, meta["b_core"])

